# revision 14
# baseline (speedup 1.0000x reference)
"""T5-style encoder block (RMSNorm -> MHA w/ relative bias -> residual ->
RMSNorm -> FFN -> residual) on 8 Trainium2 NeuronCores.

Sharding: core c handles batch b = c // 4 and query-chunk qc = c % 4
(512 queries). Each core computes K/V for its batch's full sequence
(replicated within the 4-core batch group) and attention + FFN for its
512-token chunk. No collectives.

On-chip layout is feature-major ("xT" = [D, L]): activations live with
the feature dim on SBUF partitions so projection matmuls need no
transposes. Attention scores are computed transposed ([k, q]) so the
softmax probabilities feed the attn@V matmul directly; the softmax
denominator comes from an extra ones-column appended per head to the
token-major V tiles (row 64 of the [65, 512] attn output PSUM = sum of
probs). Softmax skips the max-subtraction (scores are bounded ~ +-25
for this distribution, safe in fp32). The T5 relative-position bias is
Toeplitz in (k - q), so each core gets a host-built shifted-diagonal
matrix C[h, p, m] = bias_diag[h, p - m + 3967 - 512*qc]; the bias tile
for any (head, k-tile) is just a column slice of C.

Weights are pre-tiled on the host so every weight DMA lands 2KB+
contiguous runs per partition row (the DMA engines pay 2x below 512B).

Matmuls run in bf16 (fp32 PSUM accumulation); norms/softmax stay fp32.
"""

import math
import numpy as np
from ml_dtypes import bfloat16

import concourse.bass as bass
import concourse.bacc as bacc
import concourse.mybir as mybir
from concourse import tile
from concourse.bass_utils import run_bass_kernel_spmd

AFT = mybir.ActivationFunctionType
F32, BF = mybir.dt.float32, mybir.dt.bfloat16

B, L, D, H, HD, DFF = 2, 2048, 1024, 16, 64, 4096
NUM_BUCKETS, MAX_DISTANCE = 32, 128
CH = 512          # tokens per core (query chunk)
ND = D // 128     # 8 feature tiles
NF = DFF // 128   # 32 dff tiles
NKT = L // 128    # 16 key-token tiles
NC_ = 8           # cores
CW = 2432         # width of the shifted bias matrix C
VW = H * 65       # 1040: V token tiles, 65 cols/head (64 vals + ones)
EPS = 1e-6

_CACHE = {}


def _build_program(repeats=1, sim_local_cc=False):
    nc = bacc.Bacc("TRN2", target_bir_lowering=False, debug=False, num_devices=NC_)

    xTq = nc.dram_tensor("xTq", [D, CH], F32, kind="ExternalInput").ap()
    # pre-tiled weights: wX[j][p, 128*i + c] = WX[128*i + p, 128*j + c]
    wq = nc.dram_tensor("wq", [ND, 128, D], BF, kind="ExternalInput").ap()
    wk = nc.dram_tensor("wk", [ND, 128, D], BF, kind="ExternalInput").ap()
    wo = nc.dram_tensor("wo", [ND, 128, D], BF, kind="ExternalInput").ap()
    wv = nc.dram_tensor("wv", [ND, 128, D], BF, kind="ExternalInput").ap()   # wv[i] = Wv[128i:+128, :]
    w1 = nc.dram_tensor("w1", [NF, 128, D], BF, kind="ExternalInput").ap()   # w1[f][p, 128i+c] = W1[128i+p, 128f+c]
    w2 = nc.dram_tensor("w2", [NF, 128, D], BF, kind="ExternalInput").ap()   # w2[f] = W2[128f:+128, :]
    cb = nc.dram_tensor("cb", [H, 128, CW], BF, kind="ExternalInput").ap()
    outT = nc.dram_tensor("outT", [D, CH], F32, kind="ExternalOutput").ap()
    # combined K+V gather payload: slot 0..7 = kc[j] [128,CH]; 8..11 = vc[tt] [128,D] viewed as 2x[128,CH]
    kg_in = nc.dram_tensor("kg_in", [ND + 8, 128, CH], BF)
    kg_out = nc.dram_tensor("kg_out", [4, ND + 8, 128, CH], BF)

    with tile.TileContext(nc) as tc:
      for _rep in range(repeats):
        with tc.tile_pool(name=f"persist{_rep}", bufs=1) as pp:
            ones = pp.tile([128, 1], F32, tag="ones", name="ones")
            nc.gpsimd.memset(ones[:], 1.0)
            epsc = pp.tile([1, 1], F32, tag="epsc", name="epsc")
            nc.gpsimd.memset(epsc[:], EPS)
            hTq = [pp.tile([128, CH], BF, tag=f"hTq{i}", name=f"hTq{i}") for i in range(ND)]
            qT = [pp.tile([128, CH], BF, tag=f"qT{j}", name=f"qT{j}") for j in range(ND)]

            with tc.tile_pool(name=f"kv{_rep}", bufs=1) as kvp:
                kT = [kvp.tile([128, L], BF, tag=f"kT{j}", name=f"kT{j}") for j in range(ND)]
                vt = [kvp.tile([128, VW], BF, tag=f"vt{t}", name=f"vt{t}") for t in range(NKT)]

                # ---------------- phase 0/1: chunk rmsnorm + Q/K/V of the
                # chunk; K/V all-gathered across the 4-core batch group
                with tc.tile_pool(name=f"ph01{_rep}", bufs=1) as hp, \
                     tc.tile_pool(name=f"xs{_rep}", bufs=9) as xsp, \
                     tc.tile_pool(name=f"sq{_rep}", bufs=2) as sqp, \
                     tc.tile_pool(name=f"ws{_rep}", bufs=3) as wsp, \
                     tc.tile_pool(name=f"ev{_rep}", bufs=2) as evp, \
                     tc.tile_pool(name=f"mm{_rep}", bufs=4, space="PSUM") as mmp:

                    wv_sb = [hp.tile([128, D], BF, tag=f"wv{i}", name=f"wv{i}") for i in range(ND)]
                    S = hp.tile([128, 512], F32, tag="S", name="S")

                    # rmsnorm of the query chunk (the only tokens this core
                    # projects; K/V for the rest arrive via all-gather)
                    xts = []
                    vps = mmp.tile([1, 512], F32, tag="var", name="varq", bufs=2)
                    for i in range(ND):
                        xt = xsp.tile([128, 512], F32, tag="xs", name=f"xq_{i}")
                        nc.sync.dma_start(xt[:], xTq[128 * i:128 * (i + 1), :])
                        xts.append(xt)
                        sq = sqp.tile([128, 512], F32, tag="sq", name=f"sqq_{i}")
                        nc.vector.tensor_mul(sq[:], xt[:], xt[:])
                        nc.tensor.matmul(vps[:], ones[:], sq[:], start=(i == 0), stop=(i == ND - 1))
                    std = evp.tile([1, 512], F32, tag="std", name="stdq")
                    nc.scalar.activation(std[:], vps[:], AFT.Sqrt, bias=epsc[:], scale=1.0 / D)
                    srow = evp.tile([1, 512], F32, tag="srow", name="srowq")
                    nc.vector.reciprocal(srow[:], std[:])
                    nc.gpsimd.partition_broadcast(S[:], srow[:])
                    for i in range(ND):
                        nc.vector.tensor_mul(hTq[i][:], xts[i][:], S[:])

                    # K of the chunk: kc[j] = (Wk col-block j)^T hTq -> kg_in
                    for j in range(ND):
                        wt = wsp.tile([128, D], BF, tag="w", name=f"wkb{j}")
                        nc.sync.dma_start(wt[:], wk[j])
                        ps = mmp.tile([128, 512], F32, tag="acc", name=f"kps{j}")
                        for i in range(ND):
                            nc.tensor.matmul(ps[:], wt[:, 128 * i:128 * (i + 1)], hTq[i][:],
                                             start=(i == 0), stop=(i == ND - 1))
                        kc = sqp.tile([128, CH], BF, tag="kc", name=f"kc{j}", bufs=3)
                        nc.vector.tensor_copy(kc[:], ps[:])
                        nc.sync.dma_start(kg_in[j], kc[:])

                    # V of the chunk, token-major [128 tok, D] -> vg_in
                    for i in range(ND):
                        nc.sync.dma_start(wv_sb[i][:], wv[i])
                    for tt in range(4):
                        vc = hp.tile([128, D], BF, tag=f"vc{tt}", name=f"vc{tt}")
                        for co in range(2):
                            ps = mmp.tile([128, 512], F32, tag="acc", name=f"vps{tt}_{co}")
                            for i in range(ND):
                                nc.tensor.matmul(ps[:], hTq[i][:, 128 * tt:128 * (tt + 1)],
                                                 wv_sb[i][:, 512 * co:512 * (co + 1)],
                                                 start=(i == 0), stop=(i == ND - 1))
                            nc.scalar.activation(vc[:, 512 * co:512 * (co + 1)], ps[:], AFT.Copy)
                        nc.sync.dma_start(kg_in[ND + 2 * tt], vc[:, 0:CH])
                        nc.sync.dma_start(kg_in[ND + 2 * tt + 1], vc[:, CH:D])

                    # Q projection (chunk only)
                    for j in range(ND):
                        wt = wsp.tile([128, D], BF, tag="w", name=f"wqb{j}")
                        nc.sync.dma_start(wt[:], wq[j])
                        ps = mmp.tile([128, 512], F32, tag="acc", name=f"qps{j}")
                        for i in range(ND):
                            nc.tensor.matmul(ps[:], wt[:, 128 * i:128 * (i + 1)], hTq[i][:],
                                             start=(i == 0), stop=(i == ND - 1))
                        nc.vector.tensor_copy(qT[j][:], ps[:])

                    # all-gather K and V chunks across the batch group
                    if sim_local_cc:
                        # TimelineSim can't run collectives; approximate the
                        # local traffic with dram-to-dram copies
                        for g in range(4):
                            nc.sync.dma_start(kg_out[g], kg_in[:])
                    else:
                        nc.gpsimd.collective_compute(
                            "AllGather", mybir.AluOpType.bypass,
                            replica_groups=[[0, 1, 2, 3], [4, 5, 6, 7]],
                            ins=[kg_in[:]], outs=[kg_out[:]])
                    for g in range(4):
                        for j in range(ND):
                            nc.sync.dma_start(kT[j][:, CH * g:CH * (g + 1)], kg_out[g, j])
                        for tt in range(4):
                            t = 4 * g + tt
                            vtr = vt[t].rearrange("p (h c) -> p h c", c=65)
                            nc.gpsimd.memset(vtr[:, :, 64:65], 1.0)
                            nc.sync.dma_start(vtr[:, 0:8, 0:64], kg_out[g, ND + 2 * tt])
                            nc.sync.dma_start(vtr[:, 8:16, 0:64], kg_out[g, ND + 2 * tt + 1])

                # ---------------- phases 2-5: attention, out-proj, FFN
                with tc.tile_pool(name=f"mid{_rep}", bufs=1) as midp:
                    aoT = [midp.tile([128, CH], BF, tag=f"aoT{i}", name=f"aoT{i}") for i in range(ND)]
                    x2T = [midp.tile([128, CH], F32, tag=f"x2T{i}", name=f"x2T{i}") for i in range(ND)]
                    h2T = [midp.tile([128, CH], BF, tag=f"h2T{i}", name=f"h2T{i}") for i in range(ND)]

                    # ----- attention
                    with tc.tile_pool(name=f"cp{_rep}", bufs=2) as cp, \
                         tc.tile_pool(name=f"ppool{_rep}", bufs=4) as ppool, \
                         tc.tile_pool(name=f"rp{_rep}", bufs=2) as rp, \
                         tc.tile_pool(name=f"scp{_rep}", bufs=2, space="PSUM") as scp, \
                         tc.tile_pool(name=f"aop{_rep}", bufs=2, space="PSUM") as aop:
                        for hp in range(H // 2):
                            # head pair (2*hp, 2*hp+1): both live in kT[hp];
                            # one [128,1024] score tile -> one exp, one mult
                            j = hp
                            ch = cp.tile([128, 2 * CW], BF, tag="C", name=f"C{hp}")
                            chr_ = ch.rearrange("p (hh w) -> p hh w", w=CW)
                            nc.sync.dma_start(chr_[:, 0:1, :], cb[2 * hp])
                            nc.sync.dma_start(chr_[:, 1:2, :], cb[2 * hp + 1])
                            aops = [aop.tile([65, 512], F32, tag="ao", name=f"aops{hp}_{hh}", bufs=4)
                                    for hh in range(2)]
                            for kt in range(NKT):
                                sc = scp.tile([128, 1024], F32, tag="sc", name=f"sc{hp}_{kt}")
                                for hh in range(2):
                                    nc.tensor.matmul(sc[:, 512 * hh:512 * (hh + 1)],
                                                     kT[j][64 * hh:64 * (hh + 1), 128 * kt:128 * (kt + 1)],
                                                     qT[j][64 * hh:64 * (hh + 1), :], start=True, stop=True)
                                es = ppool.tile([128, 1024], BF, tag="es", name=f"es{hp}_{kt}")
                                nc.scalar.activation(es[:], sc[:], AFT.Exp)
                                p = ppool.tile([128, 1024], BF, tag="p", name=f"p{hp}_{kt}")
                                nc.vector.tensor_mul(p.rearrange("q (hh c) -> q hh c", c=512),
                                                     es.rearrange("q (hh c) -> q hh c", c=512),
                                                     chr_[:, :, 1920 - 128 * kt:2432 - 128 * kt])
                                vtr = vt[kt].rearrange("q (hh c) -> q hh c", c=65)
                                for hh in range(2):
                                    nc.tensor.matmul(aops[hh][:], vtr[:, 2 * hp + hh:2 * hp + hh + 1, :],
                                                     p[:, 512 * hh:512 * (hh + 1)],
                                                     start=(kt == 0), stop=(kt == NKT - 1))
                            for hh in range(2):
                                r0 = 64 * hh
                                rrow = rp.tile([1, 512], F32, tag="rrow", name=f"rrow{hp}_{hh}")
                                nc.vector.reciprocal(rrow[:], aops[hh][64:65, :])
                                rb = rp.tile([64, 512], F32, tag="rb", name=f"rb{hp}_{hh}")
                                nc.gpsimd.partition_broadcast(rb[:], rrow[:])
                                nc.vector.tensor_mul(aoT[j][r0:r0 + 64, :], aops[hh][0:64, :], rb[:])

                    # ----- output projection + residual
                    with tc.tile_pool(name=f"ws2{_rep}", bufs=3) as wsp2, \
                         tc.tile_pool(name=f"xq{_rep}", bufs=2) as xqp, \
                         tc.tile_pool(name=f"mm2{_rep}", bufs=2, space="PSUM") as mm2:
                        for j in range(ND):
                            wt = wsp2.tile([128, D], BF, tag="w", name=f"wob{j}")
                            nc.sync.dma_start(wt[:], wo[j])
                            ps = mm2.tile([128, 512], F32, tag="o", name=f"ops{j}")
                            for i in range(ND):
                                nc.tensor.matmul(ps[:], wt[:, 128 * i:128 * (i + 1)], aoT[i][:],
                                                 start=(i == 0), stop=(i == ND - 1))
                            xqt = xqp.tile([128, 512], F32, tag="xq", name=f"xq{j}")
                            nc.sync.dma_start(xqt[:], xTq[128 * j:128 * (j + 1), :])
                            nc.vector.tensor_add(x2T[j][:], ps[:], xqt[:])

                    # ----- rmsnorm 2
                    with tc.tile_pool(name=f"n2{_rep}", bufs=1) as n2p, \
                         tc.tile_pool(name=f"sq2{_rep}", bufs=2) as sqp2, \
                         tc.tile_pool(name=f"mm3{_rep}", bufs=2, space="PSUM") as mm3:
                        S2 = n2p.tile([128, CH], F32, tag="S2", name="S2")
                        vps2 = mm3.tile([1, 512], F32, tag="var2", name="var2")
                        for i in range(ND):
                            sq = sqp2.tile([128, 512], F32, tag="sq", name=f"sq2_{i}")
                            nc.vector.tensor_mul(sq[:], x2T[i][:], x2T[i][:])
                            nc.tensor.matmul(vps2[:], ones[:], sq[:], start=(i == 0), stop=(i == ND - 1))
                        std2 = n2p.tile([1, 512], F32, tag="std2", name="std2")
                        nc.scalar.activation(std2[:], vps2[:], AFT.Sqrt, bias=epsc[:], scale=1.0 / D)
                        S2row = n2p.tile([1, CH], F32, tag="S2row", name="S2row")
                        nc.vector.reciprocal(S2row[:], std2[:])
                        nc.gpsimd.partition_broadcast(S2[:], S2row[:])
                        for i in range(ND):
                            nc.vector.tensor_mul(h2T[i][:], x2T[i][:], S2[:])

                    # ----- FFN: u = relu(W1^T h2) kept resident (bf16);
                    # out accumulates in PSUM over all 32 f-tiles, 4 dout
                    # tiles at a time (pass 0 overlaps u production)
                    with tc.tile_pool(name=f"uT{_rep}", bufs=1) as utp, \
                         tc.tile_pool(name=f"w1p{_rep}", bufs=3) as w1p, \
                         tc.tile_pool(name=f"w2p{_rep}", bufs=3) as w2p, \
                         tc.tile_pool(name=f"op{_rep}", bufs=2) as op, \
                         tc.tile_pool(name=f"up{_rep}", bufs=2, space="PSUM") as upp, \
                         tc.tile_pool(name=f"o2p{_rep}", bufs=4, space="PSUM") as o2p:
                        uT = [utp.tile([128, CH], BF, tag=f"uT{f}", name=f"uT{f}") for f in range(NF)]
                        for dh in range(2):
                            ps2 = [o2p.tile([128, 512], F32, tag="o2", name=f"o2_{dh}_{jj}") for jj in range(4)]
                            for f in range(NF):
                                if dh == 0:
                                    wt = w1p.tile([128, D], BF, tag="w1", name=f"w1b{f}")
                                    nc.sync.dma_start(wt[:], w1[f])
                                    ups = upp.tile([128, 512], F32, tag="u", name=f"u{f}")
                                    for i in range(ND):
                                        nc.tensor.matmul(ups[:], wt[:, 128 * i:128 * (i + 1)], h2T[i][:],
                                                         start=(i == 0), stop=(i == ND - 1))
                                    nc.scalar.activation(uT[f][:], ups[:], AFT.Relu)
                                wt2 = w2p.tile([128, 512], BF, tag="w2", name=f"w2b{dh}_{f}")
                                nc.sync.dma_start(wt2[:], w2[f][:, 512 * dh:512 * (dh + 1)])
                                for jj in range(4):
                                    nc.tensor.matmul(ps2[jj][:], wt2[:, 128 * jj:128 * (jj + 1)], uT[f][:],
                                                     start=(f == 0), stop=(f == NF - 1))
                            for jj in range(4):
                                j = 4 * dh + jj
                                ot = op.tile([128, 512], F32, tag="ot", name=f"ot{dh}_{jj}")
                                nc.vector.tensor_add(ot[:], ps2[jj][:], x2T[j][:])
                                nc.sync.dma_start(outT[128 * j:128 * (j + 1), :], ot[:])

    nc.compile()
    return nc


def _bias_diag(rel_table):
    """bias_diag[h, i] = bias for relative position d = i - (L-1), i in [0, 2L-1)."""
    d = np.arange(-(L - 1), L)
    nb = NUM_BUCKETS // 2
    buckets = (d > 0).astype(np.int64) * nb
    rpa = np.abs(d)
    max_exact = nb // 2
    is_small = rpa < max_exact
    safe = np.maximum(rpa, 1).astype(np.float32)
    large = max_exact + (
        np.log(safe / max_exact) / math.log(MAX_DISTANCE / max_exact) * (nb - max_exact)
    ).astype(np.int64)
    large = np.minimum(large, nb - 1)
    buckets = buckets + np.where(is_small, rpa, large)
    return np.ascontiguousarray(rel_table[buckets].T.astype(np.float32))  # [H, 2L-1]


def _colblocks(w):
    """[D_in, N] -> [N//128, 128, D_in] with out[j][p, 128*i + c] = w[128*i + p, 128*j + c].

    One DMA per 128-wide output column block; each SBUF partition row is a
    2KB+ contiguous run in DRAM.
    """
    din, n = w.shape
    # [i, p, j, c] -> [j, p, i, c]
    t = w.reshape(din // 128, 128, n // 128, 128).transpose(2, 1, 0, 3)
    return np.ascontiguousarray(t.reshape(n // 128, 128, din))


def kernel(hidden_states, Wq, Wk, Wv, Wo, W1, W2, ln1_g, ln2_g, rel_table):
    if "nc" not in _CACHE:
        _CACHE["nc"] = _build_program()
    nc = _CACHE["nc"]

    x = np.asarray(hidden_states, dtype=np.float32)
    g1c = np.asarray(ln1_g, dtype=np.float32)[:, None]   # fold gains into weights
    g2c = np.asarray(ln2_g, dtype=np.float32)[:, None]
    wq_t = _colblocks((np.asarray(Wq, dtype=np.float32) * g1c).astype(bfloat16))
    wk_t = _colblocks((np.asarray(Wk, dtype=np.float32) * g1c).astype(bfloat16))
    wo_t = _colblocks(np.asarray(Wo, dtype=bfloat16))
    wv_t = np.ascontiguousarray((np.asarray(Wv, dtype=np.float32) * g1c).astype(bfloat16).reshape(ND, 128, D))
    w1_t = _colblocks((np.asarray(W1, dtype=np.float32) * g2c).astype(bfloat16))
    w2_t = np.ascontiguousarray(np.asarray(W2, dtype=bfloat16).reshape(NF, 128, D))

    bias_diag = np.exp(_bias_diag(np.asarray(rel_table, dtype=np.float32)))  # [H, 4095], exp'd
    p_idx = np.arange(128)[:, None]
    m_idx = np.arange(CW)[None, :]

    in_maps = []
    for c in range(NC_):
        b, qc = c // 4, c % 4
        xTq = np.ascontiguousarray(x[b, qc * CH:(qc + 1) * CH].T)
        idx = p_idx - m_idx + (3967 - 512 * qc)
        cb_c = np.ascontiguousarray(bias_diag[:, idx].astype(bfloat16))  # [H,128,CW]
        in_maps.append({
            "xTq": xTq,
            "wq": wq_t, "wk": wk_t, "wv": wv_t, "wo": wo_t,
            "w1": w1_t, "w2": w2_t, "cb": cb_c,
        })

    res = run_bass_kernel_spmd(nc, in_maps, list(range(NC_)))

    out = np.empty((B, L, D), dtype=np.float32)
    for c in range(NC_):
        b, qc = c // 4, c % 4
        out[b, qc * CH:(qc + 1) * CH, :] = res.results[c]["outT"].T
    return out
